# revision 1
# baseline (speedup 1.0000x reference)
"""Trainium2 Bass kernel for 16-head MHA with RoPE (dense_transformer).

Sharding: tensor-parallel over heads (2 heads/core on 8 cores) for
QKV projection + attention, then an AllToAll resharding to
token-parallel (512 tokens/core) for the output projection.

Layout strategy (per core, rank r):
  - x arrives column-sliced ([4096, 128] slice r); the core PE-transposes
    its slice and an AllGather materializes xT [1024, 4096] (dim-major
    activations) in DRAM, shared work across cores.
  - qT/kT/vT [128, 4096] are computed dim-major (feature dim on
    partitions) via  wT.T @ xT  matmuls in float32r.
  - RoPE is fused into the PSUM->SBUF evacuation: one tensor_tensor
    multiply by cos, one by a sign-folded sin table, plus 4 strip
    SBUF->SBUF accumulate-DMAs that implement rotate_half's partition
    rotation.
  - v is re-transposed to token-major [keys, 64] tiles with a ones
    column appended, so attention  out.T = [v | 1].T @ exp(S.T)  yields
    the softmax denominator as row 64 for free.
  - Scores are computed transposed (S.T = k.T^T @ qT per 128-key chunk),
    exp on ScalarE with the 1/sqrt(hd) scale folded in; no max
    subtraction (scores are bounded ~|9.3|, exp stays finite in fp32).
  - AllToAll reshards attention output from head-parallel to
    token-parallel; o-projection consumes the gathered aT dim-major.
"""

import numpy as np

# Problem shape (hardcoded per contract - kernel.py must be self-contained)
B, L_FULL, D = 2, 2048, 1024
H, HD = 16, 64
N_CORES = 8
HPC = H // N_CORES            # heads per core = 2
DPC = D // N_CORES            # xT dim-slice per core = 128
KC = D // 128                 # contraction chunks = 8


def _rope_tables(L):
    inv_freq = 1.0 / (10000.0 ** (np.arange(0, HD, 2, dtype=np.float64) / HD))
    t = np.arange(L, dtype=np.float64)
    freqs = np.outer(t, inv_freq)                      # [L, 32]
    emb = np.concatenate([freqs, freqs], -1)           # [L, 64]
    cos_t = np.cos(emb).T.astype(np.float32)           # [64, L]
    sin_t = np.sin(emb).T.astype(np.float32)
    cost = np.concatenate([cos_t, cos_t], 0)           # [128, L] (2 heads)
    sp = np.concatenate([sin_t[:32], -sin_t[32:]], 0)  # sign-folded
    sinp = np.concatenate([sp, sp], 0)                 # [128, L]
    return np.ascontiguousarray(cost), np.ascontiguousarray(sinp)


def build_mha(tc, L=L_FULL, debug=False):
    """Emit the MHA program into TileContext `tc`.

    Declares its own DRAM I/O tensors:
      in : xcol [B*L, 128], wqt/wkt/wvt [D, 128], wot [D, D]
      out: y [B*L/8, D]
    """
    import concourse.bass as bass
    import concourse.mybir as mybir
    from contextlib import ExitStack

    nc = tc.nc
    f32 = mybir.dt.float32
    f32r = mybir.dt.float32r
    AF = mybir.ActivationFunctionType
    ALU = mybir.AluOpType

    T = B * L                     # tokens
    TPC = T // N_CORES            # tokens per core (a2a shard width)
    CH = min(512, L)              # projection token-chunk (never crosses a batch)
    NCH = T // CH                 # projection chunks
    MC = L // 128                 # key chunks per batch
    FQ = min(1024, L)             # attention query tile (exp free-dim)
    NQ = min(512, FQ)             # matmul moving-dim tile
    NH = L // FQ                  # query tiles per batch
    MT = min(128, TPC)            # o-proj token tile
    scale = float(HD) ** -0.5
    rg = [list(range(N_CORES))]

    def r(ap):
        return ap.bitcast(f32r)

    # ---- I/O ----
    xcol_d = nc.dram_tensor("xcol", [T, DPC], f32, kind="ExternalInput").ap()
    wqt_d = nc.dram_tensor("wqt", [D, 128], f32, kind="ExternalInput").ap()
    wkt_d = nc.dram_tensor("wkt", [D, 128], f32, kind="ExternalInput").ap()
    wvt_d = nc.dram_tensor("wvt", [D, 128], f32, kind="ExternalInput").ap()
    wot_d = nc.dram_tensor("wot", [D, D], f32, kind="ExternalInput").ap()
    y_d = nc.dram_tensor("y", [TPC, D], f32, kind="ExternalOutput").ap()
    if debug:
        dbg_st0 = nc.dram_tensor("dbg_st0", [128, FQ], f32, kind="ExternalOutput").ap()
        dbg_st1 = nc.dram_tensor("dbg_st1", [128, FQ], f32, kind="ExternalOutput").ap()
        dbg_pt0 = nc.dram_tensor("dbg_pt0", [128, FQ], f32, kind="ExternalOutput").ap()
        dbg_pt1 = nc.dram_tensor("dbg_pt1", [128, FQ], f32, kind="ExternalOutput").ap()
        dbg_ou0 = nc.dram_tensor("dbg_ou0", [65, FQ], f32, kind="ExternalOutput").ap()
        dbg_dinv = nc.dram_tensor("dbg_dinv", [1, FQ], f32, kind="ExternalOutput").ap()
        dbg_ast = nc.dram_tensor("dbg_ast", [64, FQ], f32, kind="ExternalOutput").ap()
        dbg_qt = nc.dram_tensor("dbg_qt", [128, T], f32, kind="ExternalOutput").ap()
        dbg_kt = nc.dram_tensor("dbg_kt", [128, T], f32, kind="ExternalOutput").ap()
        dbg_vt = nc.dram_tensor("dbg_vt", [128, T], f32, kind="ExternalOutput").ap()
        dbg_ai = nc.dram_tensor("dbg_ai", [D, TPC], f32, kind="ExternalOutput").ap()
        dbg_ao = nc.dram_tensor("dbg_ao", [D, TPC], f32, kind="ExternalOutput").ap()
        dbg_ag = nc.dram_tensor("dbg_ag", [D, T], f32, kind="ExternalOutput").ap()

    # ---- inline constants ----
    cost_np, sinp_np = _rope_tables(L)
    ident_d = nc.inline_tensor(np.eye(128, dtype=np.float32), name="ident")
    cost_d = nc.inline_tensor(cost_np, name="cost")
    sinp_d = nc.inline_tensor(sinp_np, name="sinp")
    ones_d = nc.inline_tensor(np.ones((1, 64), dtype=np.float32), name="ones64")

    ctx = ExitStack()
    with ctx:
        # ---------------- persistent pools ----------------
        cpool = ctx.enter_context(tc.tile_pool(name="consts", bufs=1))
        ident = cpool.tile([128, 128], f32)
        nc.sync.dma_start(ident[:], ident_d.ap()[:, :])
        cost = cpool.tile([128, L], f32)
        nc.sync.dma_start(cost[:], cost_d.ap()[:, :])
        sinp = cpool.tile([128, L], f32)
        nc.sync.dma_start(sinp[:], sinp_d.ap()[:, :])
        ones64 = cpool.tile([1, 64], f32)
        nc.sync.dma_start(ones64[:], ones_d.ap()[:, :])

        dram = ctx.enter_context(tc.tile_pool(name="dram", bufs=1, space="DRAM"))
        ag_in = dram.tile([DPC, T], f32)
        ag_out = dram.tile([D, T], f32, addr_space="Shared")
        a2a_in = dram.tile([D, TPC], f32)
        a2a_out = dram.tile([D, TPC], f32)

        qkpool = ctx.enter_context(tc.tile_pool(name="qk", bufs=1))
        qt = qkpool.tile([128, T], f32)   # post-RoPE q, dim-major (f32 accum)
        kt = qkpool.tile([128, T], f32)
        qtr = qkpool.tile([128, T], f32r)  # rounded copies feeding matmuls
        ktr = qkpool.tile([128, T], f32r)

        # ---------------- stage 1: xT slice + AllGather ----------------
        with tc.tile_pool(name="xtr", bufs=1) as xtrp, \
             tc.tile_pool(name="xc", bufs=3) as xcp, \
             tc.tile_pool(name="tps", bufs=2, space="PSUM") as tpsp:
            xtr = xtrp.tile([DPC, T], f32)
            for c in range(T // 128):
                xc = xcp.tile([128, DPC], f32)
                nc.sync.dma_start(xc[:], xcol_d[c * 128:(c + 1) * 128, :])
                tps = tpsp.tile([DPC, 128], f32)
                nc.tensor.transpose(tps[:], xc[:], ident[:DPC, :128])
                nc.vector.tensor_copy(xtr[:, c * 128:(c + 1) * 128], tps[:])
            nc.sync.dma_start(ag_in[:, :], xtr[:])
        nc.gpsimd.collective_compute(
            "AllGather", ALU.bypass, ins=[ag_in.opt()], outs=[ag_out.opt()],
            replica_groups=rg,
        )

        # ---------------- stage 2+3: weights + projections ----------------
        vt_pool = ctx.enter_context(tc.tile_pool(name="vtp", bufs=1))
        vt = vt_pool.tile([128, T], f32)  # v dim-major (pre-transpose)

        with tc.tile_pool(name="wqkv", bufs=1) as wp, \
             tc.tile_pool(name="xt", bufs=2) as xtp, \
             tc.tile_pool(name="u", bufs=2) as up, \
             tc.tile_pool(name="pps", bufs=2, space="PSUM") as pps:
            wq_sb = wp.tile([128, KC, 128], f32r)
            wk_sb = wp.tile([128, KC, 128], f32r)
            wv_sb = wp.tile([128, KC, 128], f32r)
            for kk in range(KC):
                nc.sync.dma_start(wq_sb[:, kk, :],
                                  r(wqt_d[kk * 128:(kk + 1) * 128, :]))
                nc.sync.dma_start(wk_sb[:, kk, :],
                                  r(wkt_d[kk * 128:(kk + 1) * 128, :]))
                nc.sync.dma_start(wv_sb[:, kk, :],
                                  r(wvt_d[kk * 128:(kk + 1) * 128, :]))

            for c in range(NCH):
                l0 = (c * CH) % L   # position within batch (tables index)
                sl = slice(c * CH, (c + 1) * CH)
                xt = xtp.tile([128, KC, CH], f32r)
                for kk in range(KC):
                    nc.sync.dma_start(
                        xt[:, kk, :], r(ag_out[kk * 128:(kk + 1) * 128, sl]))
                q_ps = pps.tile([128, CH], f32, tag="q_ps")
                k_ps = pps.tile([128, CH], f32, tag="k_ps")
                v_ps = pps.tile([128, CH], f32, tag="v_ps")
                for kk in range(KC):
                    st_, sp_ = (kk == 0), (kk == KC - 1)
                    nc.tensor.matmul(q_ps[:], r(wq_sb[:, kk, :]), r(xt[:, kk, :]),
                                     start=st_, stop=sp_)
                    nc.tensor.matmul(k_ps[:], r(wk_sb[:, kk, :]), r(xt[:, kk, :]),
                                     start=st_, stop=sp_)
                    nc.tensor.matmul(v_ps[:], r(wv_sb[:, kk, :]), r(xt[:, kk, :]),
                                     start=st_, stop=sp_)
                # RoPE-fused evacuation for q and k
                tb = slice(l0, l0 + CH)
                for ps, dst, dstr in ((q_ps, qt, qtr), (k_ps, kt, ktr)):
                    u = up.tile([128, CH], f32, tag="u")
                    nc.vector.tensor_mul(u[:], ps[:], sinp[:, tb])
                    nc.vector.tensor_mul(dst[:, sl], ps[:], cost[:, tb])
                    for h in range(HPC):
                        a, b_ = h * 64, h * 64 + 32
                        c_ = h * 64 + 64
                        nc.gpsimd.dma_start(dst[a:b_, sl], u[b_:c_, :],
                                            accum_op=ALU.add)
                        nc.gpsimd.dma_start(dst[b_:c_, sl], u[a:b_, :],
                                            accum_op=ALU.add)
                    nc.vector.tensor_copy(dstr[:, sl], dst[:, sl])
                nc.vector.tensor_copy(vt[:, sl], v_ps[:])

        # ---------------- stage 4: v -> token-major [keys, 64|1] ----------
        vpool = ctx.enter_context(tc.tile_pool(name="vtm", bufs=1))
        v_sb = [vpool.tile([128, HPC, MC, 65], f32r, tag=f"v{b}", name=f"v_sb{b}")
                for b in range(B)]
        with tc.tile_pool(name="vps", bufs=2, space="PSUM") as vps, \
             tc.tile_pool(name="onc", bufs=1) as onc:
            ones_col = onc.tile([128, HPC, MC, 1], f32)
            nc.gpsimd.memset(ones_col[:], 1.0)
            for b in range(B):
                nc.vector.tensor_copy(v_sb[b][:, :, :, 64:65], ones_col[:])
                for h in range(HPC):
                    hs = slice(h * 64, (h + 1) * 64)
                    for m in range(MC):
                        ks = slice(b * L + m * 128, b * L + (m + 1) * 128)
                        vp = vps.tile([128, 64], f32)
                        nc.tensor.transpose(vp[:], vt[hs, ks], ident[hs, hs])
                        nc.vector.tensor_copy(v_sb[b][:, h, m, 0:64], vp[:])

        # ---------------- stage 5: attention ----------------
        s5 = ExitStack()
        epool = s5.enter_context(tc.tile_pool(name="ep", bufs=2))
        ptpool = s5.enter_context(tc.tile_pool(name="pt", bufs=2))
        stp = s5.enter_context(tc.tile_pool(name="stp", bufs=1, space="PSUM"))
        oup = s5.enter_context(tc.tile_pool(name="oup", bufs=1, space="PSUM"))

        for b in range(B):
            for nh in range(NH):
                q0 = b * L + nh * FQ          # global query offset
                outU = [oup.tile([65, FQ], f32, tag=f"outU{h}", name=f"outU{h}")
                        for h in range(HPC)]
                for m in range(MC):
                    ks = slice(b * L + m * 128, b * L + (m + 1) * 128)
                    sts = []
                    for h in range(HPC):
                        hs = slice(h * 64, (h + 1) * 64)
                        st = stp.tile([128, FQ], f32, tag=f"st{h}")
                        for n in range(FQ // NQ):
                            qs = slice(q0 + n * NQ, q0 + (n + 1) * NQ)
                            nc.tensor.matmul(st[:, n * NQ:(n + 1) * NQ],
                                             ktr[hs, ks], qtr[hs, qs],
                                             start=True, stop=True)
                        sts.append(st)
                    if debug and b == 0 and nh == 0 and m == 0:
                        for st_, d_ in ((sts[0], dbg_st0), (sts[1], dbg_st1)):
                            cp = epool.tile([128, FQ], f32, tag="dbgcp", name="cp")
                            nc.vector.tensor_copy(cp[:], st_[:])
                            nc.sync.dma_start(d_[:, :], cp[:])
                    pts = []
                    for h in range(HPC):
                        pt = ptpool.tile([128, FQ], f32r, tag=f"pt{h}")
                        nc.scalar.activation(pt[:], sts[h][:], AF.Exp, scale=scale)
                        pts.append(pt)
                    if debug and b == 0 and nh == 0 and m == 0:
                        nc.sync.dma_start(dbg_pt0[:, :], pts[0][:].bitcast(f32))
                        nc.sync.dma_start(dbg_pt1[:, :], pts[1][:].bitcast(f32))
                    for h in range(HPC):
                        for n in range(FQ // NQ):
                            ns = slice(n * NQ, (n + 1) * NQ)
                            nc.tensor.matmul(outU[h][:, ns],
                                             r(v_sb[b][:, h, m, :]), r(pts[h][:, ns]),
                                             start=(m == 0), stop=(m == MC - 1))
                # epilogue: normalize + stage into a2a input
                if debug and b == 0 and nh == 0:
                    cp2 = epool.tile([65, FQ], f32, tag="dbgcp2", name="cp2")
                    nc.vector.tensor_copy(cp2[:], outU[0][:])
                    nc.sync.dma_start(dbg_ou0[:, :], cp2[:])
                for h in range(HPC):
                    dinv = epool.tile([1, FQ], f32, tag="dinv")
                    nc.vector.reciprocal(dinv[:], outU[h][64:65, :])
                    if debug and b == 0 and nh == 0 and h == 0:
                        nc.sync.dma_start(dbg_dinv[:, :], dinv[:])
                    bc_sb = epool.tile([64, FQ], f32, tag="bc_sb")
                    nc.gpsimd.partition_broadcast(bc_sb[:], dinv[:])
                    a_st = epool.tile([64, FQ], f32, tag="a_st")
                    nc.vector.tensor_mul(a_st[:], outU[h][0:64, :], bc_sb[:])
                    if debug and b == 0 and nh == 0 and h == 0:
                        nc.sync.dma_start(dbg_ast[:, :], a_st[:])
                    for nq in range(FQ // TPC):
                        j = (b * L + nh * FQ) // TPC + nq
                        nc.sync.dma_start(
                            a2a_in[j * 128 + h * 64: j * 128 + (h + 1) * 64, :],
                            a_st[:, nq * TPC:(nq + 1) * TPC])

        s5.close()

        if debug:
            nc.sync.dma_start(dbg_qt[:, :], qt[:])
            nc.sync.dma_start(dbg_kt[:, :], kt[:])
            nc.sync.dma_start(dbg_vt[:, :], vt[:])
            nc.sync.dma_start(dbg_ai[:, :], a2a_in[:])
            nc.sync.dma_start(dbg_ag[:, :], ag_out[:])

        # ---------------- stage 6: AllToAll ----------------
        nc.gpsimd.collective_compute(
            "AllToAll", ALU.bypass, ins=[a2a_in.opt()], outs=[a2a_out.opt()],
            replica_groups=rg,
        )

        if debug:
            nc.sync.dma_start(dbg_ao[:, :], a2a_out[:])

        # ---------------- stage 7: o-projection ----------------
        with tc.tile_pool(name="wo", bufs=1) as wop, \
             tc.tile_pool(name="at", bufs=1) as atp, \
             tc.tile_pool(name="ysb", bufs=2) as ysp, \
             tc.tile_pool(name="yps", bufs=2, space="PSUM") as yps:
            wo_sb = wop.tile([128, KC, D], f32r)
            at_sb = atp.tile([128, KC, TPC], f32r)
            for kk in range(KC):
                nc.sync.dma_start(wo_sb[:, kk, :],
                                  r(wot_d[kk * 128:(kk + 1) * 128, :]))
                nc.sync.dma_start(at_sb[:, kk, :],
                                  r(a2a_out[kk * 128:(kk + 1) * 128, :]))
            for mt in range(TPC // MT):
                ms = slice(mt * MT, (mt + 1) * MT)
                y_sb = ysp.tile([MT, D], f32, tag="y_sb")
                for no in range(D // 512):
                    y_ps = yps.tile([MT, 512], f32, tag="y_ps")
                    for kk in range(KC):
                        nc.tensor.matmul(y_ps[:], r(at_sb[:, kk, ms]),
                                         r(wo_sb[:, kk, no * 512:(no + 1) * 512]),
                                         start=(kk == 0), stop=(kk == KC - 1))
                    nc.vector.tensor_copy(y_sb[:, no * 512:(no + 1) * 512], y_ps[:])
                nc.sync.dma_start(y_d[ms, :], y_sb[:])

    return nc


def make_in_maps(x, wq, wk, wv, wo, L=L_FULL):
    T = B * L
    x2 = np.ascontiguousarray(np.asarray(x, dtype=np.float32).reshape(T, D))
    wq = np.asarray(wq, dtype=np.float32)
    wk = np.asarray(wk, dtype=np.float32)
    wv = np.asarray(wv, dtype=np.float32)
    wo = np.asarray(wo, dtype=np.float32)
    wot = np.ascontiguousarray(wo.T)
    in_maps = []
    for rk in range(N_CORES):
        rows = slice(rk * 128, (rk + 1) * 128)
        in_maps.append({
            "xcol": np.ascontiguousarray(x2[:, rk * DPC:(rk + 1) * DPC]),
            "wqt": np.ascontiguousarray(wq[rows].T),
            "wkt": np.ascontiguousarray(wk[rows].T),
            "wvt": np.ascontiguousarray(wv[rows].T),
            "wot": wot,
        })
    return in_maps


_BUILT = {}


def _get_nc(L=L_FULL):
    if L not in _BUILT:
        import concourse.tile as tile
        from concourse import bacc
        nc = bacc.Bacc(num_devices=N_CORES)
        with tile.TileContext(nc) as tc:
            build_mha(tc, L=L)
        nc.compile()
        _BUILT[L] = nc
    return _BUILT[L]


def kernel(x, wq, wk, wv, wo):
    from concourse.bass_utils import run_bass_kernel_spmd
    nc = _get_nc()
    in_maps = make_in_maps(x, wq, wk, wv, wo)
    res = run_bass_kernel_spmd(nc, in_maps, core_ids=list(range(N_CORES)))
    y = np.concatenate([res.results[rk]["y"] for rk in range(N_CORES)], axis=0)
    return y.reshape(B, L_FULL, D)



# revision 31
# speedup vs baseline: 64.4643x; 64.4643x over previous
"""Trainium2 Bass kernel for 16-head MHA with RoPE (dense_transformer).

Sharding: tensor-parallel over heads (2 heads/core on 8 cores) for
QKV projection + attention, then a chunked AllToAll resharding to
token-parallel for the output projection.

Layout strategy (per core, rank r):
  - x arrives pre-transposed from the host as xT [1024, 4096] bf16
    (dim-major, full copy on every core) - no on-chip transpose or
    AllGather needed. Streamed in 512-token chunks, one merged DMA per
    chunk (rearranged access pattern covers all 8 contraction blocks).
  - qT/kT [128, 4096] are computed dim-major (2 heads x 64 dims on
    partitions) via  wT.T @ xT  bf16 matmuls accumulating in fp32 PSUM.
  - RoPE is fused into the PSUM->SBUF evacuation: tensor_tensor multiply
    by cos and a sign-folded sin table, with rotate_half's 32-partition
    rotation done by cross-partition tensor_tensor adds on DVE/Pool
    (no DMA round trips).
  - v is re-transposed to token-major [keys, 64] tiles with a ones
    column appended, so attention  out.T = [v | 1].T @ exp(S.T)  yields
    the softmax denominator as row 64 for free.
  - Scores are computed transposed (S.T = kT^T @ qT per 128-key chunk)
    in float32r, exp on ScalarE with the 1/sqrt(hd) scale folded in; no
    max subtraction (scores are bounded ~|9.3|, exp stays finite in
    fp32).
  - After each 1024-query attention block, its normalized output is
    shipped through a small bf16 AllToAll chunk (token-granule
    interleaved) that overlaps the next attention block; the bf16
    o-projection consumes each chunk as it lands.
"""

import numpy as np

# Problem shape (hardcoded per contract - kernel.py must be self-contained)
B, L_FULL, D = 2, 2048, 1024
H, HD = 16, 64
N_CORES = 8
HPC = H // N_CORES            # heads per core = 2
KC = D // 128                 # contraction chunks = 8


def _qblocks(L):
    """Attention query blocks: (batch, global q0, block len).

    The last block is split in half so its (smaller) AllToAll chunk and
    o-projection overlap the previous chunk's, shrinking the exposed tail.
    """
    FQ = min(1024, L)
    blocks = []
    for b in range(B):
        for nh in range(L // FQ):
            blocks.append((b, b * L + nh * FQ, FQ))
    if FQ >= 512:
        b, q0, fq = blocks.pop()
        blocks.append((b, q0, fq // 2))
        blocks.append((b, q0 + fq // 2, fq // 2))
    return blocks


def _rope_tables(L):
    inv_freq = 1.0 / (10000.0 ** (np.arange(0, HD, 2, dtype=np.float64) / HD))
    t = np.arange(L, dtype=np.float64)
    freqs = np.outer(t, inv_freq)                      # [L, 32]
    emb = np.concatenate([freqs, freqs], -1)           # [L, 64]
    cos_t = np.cos(emb).T.astype(np.float32)           # [64, L]
    sin_t = np.sin(emb).T.astype(np.float32)
    cost = np.concatenate([cos_t, cos_t], 0)           # [128, L] (2 heads)
    sp = np.concatenate([sin_t[:32], -sin_t[32:]], 0)  # sign-folded
    sinp = np.concatenate([sp, sp], 0)                 # [128, L]
    return np.ascontiguousarray(cost), np.ascontiguousarray(sinp)


def build_mha(tc, L=L_FULL):
    """Emit the MHA program into TileContext `tc`.

    Declares its own DRAM I/O tensors:
      in : xt [D, B*L] bf16 (full x, dim-major), wqt/wkt/wvt [D, 128]
           bf16, wot [D, D] bf16
      out: y [B*L/8, D] f32 (token-granule interleaved, see unshard_y)
    """
    import concourse.bass as bass
    import concourse.mybir as mybir
    from contextlib import ExitStack

    nc = tc.nc
    f32 = mybir.dt.float32
    f32r = mybir.dt.float32r
    bf16 = mybir.dt.bfloat16
    AF = mybir.ActivationFunctionType
    ALU = mybir.AluOpType

    T = B * L                     # tokens
    TPC = T // N_CORES            # tokens per core (output shard)
    CH = min(512, L)              # projection token-chunk (never crosses a batch)
    NCH = T // CH                 # projection chunks
    MC = L // 128                 # key chunks per batch
    FQ = min(1024, L)             # attention query tile (exp free-dim)
    NQ = min(512, FQ)             # matmul moving-dim tile
    qblocks = _qblocks(L)
    NCK = len(qblocks)            # a2a chunks (one per attention block)
    GS = [fq // N_CORES for (_, _, fq) in qblocks]
    scale = float(HD) ** -0.5
    rg = [list(range(N_CORES))]

    def r(ap):
        return ap.bitcast(f32r)

    # ---- I/O ----
    xt_d = nc.dram_tensor("xt", [D, T], bf16, kind="ExternalInput").ap()
    wqt_d = nc.dram_tensor("wqt", [D, 128], bf16, kind="ExternalInput").ap()
    wkt_d = nc.dram_tensor("wkt", [D, 128], bf16, kind="ExternalInput").ap()
    wvt_d = nc.dram_tensor("wvt", [D, 128], bf16, kind="ExternalInput").ap()
    wot_d = nc.dram_tensor("wot", [D, D], bf16, kind="ExternalInput").ap()
    y_d = nc.dram_tensor("y", [TPC, D], f32, kind="ExternalOutput").ap()

    # ---- inline constants ----
    cost_np, sinp_np = _rope_tables(L)
    ident_d = nc.inline_tensor(np.eye(128, dtype=np.float32), name="ident")
    cost_d = nc.inline_tensor(cost_np, name="cost")
    sinp_d = nc.inline_tensor(sinp_np, name="sinp")
    # rot32: per-64 block swap of 32-partition halves (symmetric permutation)
    rot_np = np.zeros((128, 128), dtype=np.float32)
    for hh in range(2):
        o = hh * 64
        rot_np[o:o + 32, o + 32:o + 64] = np.eye(32)
        rot_np[o + 32:o + 64, o:o + 32] = np.eye(32)
    rot_d = nc.inline_tensor(rot_np, name="rot32")

    ctx = ExitStack()
    with ctx:
        # ---------------- persistent SBUF pools ----------------
        # (everything fits at once; recycling SBUF between phases makes the
        # Tile arena insert cross-engine waits that serialize the phases)
        cpool = ctx.enter_context(tc.tile_pool(name="consts", bufs=1))
        ident = cpool.tile([128, 128], f32)
        cost = cpool.tile([128, L], f32)
        sinp = cpool.tile([128, L], f32)

        dram = ctx.enter_context(tc.tile_pool(name="dram", bufs=1, space="DRAM"))
        a2a_in = [dram.tile([D, GS[c]], bf16, tag=f"a2ai{c}", name=f"a2a_in{c}")
                  for c in range(NCK)]
        a2a_out = [dram.tile([D, GS[c]], bf16, tag=f"a2ao{c}", name=f"a2a_out{c}")
                   for c in range(NCK)]

        # per-batch q/k tiles: a single [128, T] tile would make batch-0
        # attention wait on batch-1's projection writes (per-tile deps)
        qkpool = ctx.enter_context(tc.tile_pool(name="qk", bufs=1))
        qt = [qkpool.tile([128, L], f32r, tag=f"qt{b}", name=f"qt{b}")
              for b in range(B)]
        kt = [qkpool.tile([128, L], f32r, tag=f"kt{b}", name=f"kt{b}")
              for b in range(B)]

        wop = ctx.enter_context(tc.tile_pool(name="wo", bufs=1))
        wo_sb = wop.tile([128, KC, D], bf16)

        vt_pool = ctx.enter_context(tc.tile_pool(name="vtp", bufs=1))
        vt = [vt_pool.tile([128, L], f32, tag=f"vt{b}", name=f"vt{b}")
              for b in range(B)]  # v dim-major (pre-transpose)
        vpool = ctx.enter_context(tc.tile_pool(name="vtm", bufs=1))
        v_sb = [vpool.tile([128, HPC, MC, 65], f32r, tag=f"v{b}", name=f"v_sb{b}")
                for b in range(B)]

        wp = ctx.enter_context(tc.tile_pool(name="wqkv", bufs=1))
        xtp = ctx.enter_context(tc.tile_pool(name="xs", bufs=2))
        up = ctx.enter_context(tc.tile_pool(name="u", bufs=2))
        onc = ctx.enter_context(tc.tile_pool(name="onc", bufs=1))
        epool = ctx.enter_context(tc.tile_pool(name="ep", bufs=2))
        ptpool = ctx.enter_context(tc.tile_pool(name="pt", bufs=2))
        atp = ctx.enter_context(tc.tile_pool(name="at", bufs=2))
        ysp = ctx.enter_context(tc.tile_pool(name="ysb", bufs=2))

        # ---------------- setup DMAs (issue order matters) ----------------
        # qkv weights + first x chunks feed the first matmuls; RoPE tables
        # are needed a few us later; ident / ones / wo later still.
        wq_sb = wp.tile([128, KC, 128], bf16)
        wk_sb = wp.tile([128, KC, 128], bf16)
        wv_sb = wp.tile([128, KC, 128], bf16)
        for w_sb, w_d in ((wq_sb, wqt_d), (wk_sb, wkt_d), (wv_sb, wvt_d)):
            nc.sync.dma_start(w_sb[:, :, :],
                              w_d[:, :].rearrange("(k p) c -> p k c", k=KC))
        xcs = []
        for c in range(2):
            xc = xtp.tile([128, KC, CH], bf16, tag="xc")
            nc.sync.dma_start(
                xc[:, :, :],
                xt_d[:, c * CH:(c + 1) * CH].rearrange(
                    "(k p) t -> p k t", k=KC))
            xcs.append(xc)
        # table halves in consumption order (chunk c uses l0 = c*CH % L)
        LH = L // 2
        nc.sync.dma_start(cost[:, 0:LH], cost_d.ap()[:, 0:LH])
        nc.sync.dma_start(sinp[:, 0:LH], sinp_d.ap()[:, 0:LH])
        nc.sync.dma_start(cost[:, LH:L], cost_d.ap()[:, LH:L])
        nc.sync.dma_start(sinp[:, LH:L], sinp_d.ap()[:, LH:L])
        nc.sync.dma_start(ident[:], ident_d.ap()[:, :])
        rot32 = wp.tile([128, 128], f32r)
        nc.sync.dma_start(rot32[:], r(rot_d.ap()[:, :]))
        ones_col = onc.tile([128, HPC, MC, 1], f32)
        nc.gpsimd.memset(ones_col[:], 1.0)
        for b in range(B):
            nc.vector.tensor_copy(v_sb[b][:, :, :, 64:65], ones_col[:])

        CPB = L // CH                 # projection chunks per batch

        # ---------------- stage 1: projections + RoPE + v transpose -------
        with tc.tile_pool(name="pps", bufs=2, space="PSUM") as pps, \
             tc.tile_pool(name="aux", bufs=2, space="PSUM") as vps:
            for c in range(NCH):
                bb = c // CPB
                l0 = (c * CH) % L   # position within batch (qt/kt cols)
                tb = slice(l0, l0 + CH)
                xc = xcs[c]
                if c + 2 < NCH:     # prefetch two chunks ahead
                    nxt = xtp.tile([128, KC, CH], bf16, tag="xc")
                    nc.sync.dma_start(
                        nxt[:, :, :],
                        xt_d[:, (c + 2) * CH:(c + 3) * CH].rearrange(
                            "(k p) t -> p k t", k=KC))
                    xcs.append(nxt)
                q_ps = pps.tile([128, CH], f32, tag="q_ps")
                k_ps = pps.tile([128, CH], f32, tag="k_ps")
                v_ps = pps.tile([128, CH], f32, tag="v_ps")
                for kk in range(KC):
                    st_, sp_ = (kk == 0), (kk == KC - 1)
                    nc.tensor.matmul(q_ps[:], wq_sb[:, kk, :], xc[:, kk, :],
                                     start=st_, stop=sp_)
                    nc.tensor.matmul(k_ps[:], wk_sb[:, kk, :], xc[:, kk, :],
                                     start=st_, stop=sp_)
                    nc.tensor.matmul(v_ps[:], wv_sb[:, kk, :], xc[:, kk, :],
                                     start=st_, stop=sp_)
                # RoPE-fused evacuation:  dst = ps*cos + rot32 @ (ps*sin')
                # (the 32-partition rotation runs as a PE permutation
                # matmul: elementwise engines cannot cross partitions)
                for ps, dst in ((q_ps, qt[bb]), (k_ps, kt[bb])):
                    u = up.tile([128, CH], f32r, tag="u")
                    nc.vector.tensor_mul(u[:], ps[:], sinp[:, tb])
                    nc.vector.tensor_mul(dst[:, tb], ps[:], cost[:, tb])
                    rps = vps.tile([128, CH], f32, tag="aux")
                    nc.tensor.matmul(rps[:], rot32[:], u[:],
                                     start=True, stop=True)
                    nc.vector.tensor_tensor(dst[:, tb], dst[:, tb],
                                            rps[:], ALU.add)
                nc.vector.tensor_copy(vt[bb][:, tb], v_ps[:])
                # once a batch's v is complete, transpose it token-major
                if (c + 1) % CPB == 0:
                    b = c // CPB
                    for m in range(MC):
                        ks = slice(m * 128, (m + 1) * 128)
                        vp = vps.tile([128, CH], f32, tag="aux")
                        nc.tensor.transpose(vp[:, 0:128], vt[b][:, ks],
                                            ident[:, :])
                        for h in range(HPC):
                            nc.vector.tensor_copy(v_sb[b][:, h, m, 0:64],
                                                  vp[:, h * 64:(h + 1) * 64])

        # ---------------- stage 2: attention + chunked a2a ----------------
        # o-proj weights: only needed once the first a2a chunk lands, and
        # the DMA engines are nearly idle during attention.
        nc.sync.dma_start(wo_sb[:, :, :],
                          wot_d[:, :].rearrange("(k p) c -> p k c", k=KC))
        s5 = ExitStack()
        stp = s5.enter_context(tc.tile_pool(name="stp", bufs=1, space="PSUM"))
        oup = s5.enter_context(tc.tile_pool(name="oup", bufs=1, space="PSUM"))

        for cchunk, (b, q0, fq) in enumerate(qblocks):
            G = GS[cchunk]
            outU = [oup.tile([65, FQ], f32, tag=f"outU{h}", name=f"outU{h}")
                    for h in range(HPC)]
            l0 = q0 - b * L           # within-batch query offset
            for m in range(MC):
                ks = slice(m * 128, (m + 1) * 128)
                sts = []
                for h in range(HPC):
                    hs = slice(h * 64, (h + 1) * 64)
                    st = stp.tile([128, FQ], f32, tag=f"st{h}")
                    for n in range(max(1, fq // NQ)):
                        qs = slice(l0 + n * NQ, l0 + min((n + 1) * NQ, fq))
                        nc.tensor.matmul(st[:, n * NQ:min((n + 1) * NQ, fq)],
                                         r(kt[b][hs, ks]), r(qt[b][hs, qs]),
                                         start=True, stop=True)
                    sts.append(st)
                pts = []
                for h in range(HPC):
                    pt = ptpool.tile([128, FQ], f32r, tag=f"pt{h}")
                    nc.scalar.activation(pt[:, 0:fq], sts[h][:, 0:fq],
                                         AF.Exp, scale=scale)
                    pts.append(pt)
                for h in range(HPC):
                    for n in range(max(1, fq // NQ)):
                        ns = slice(n * NQ, min((n + 1) * NQ, fq))
                        nc.tensor.matmul(outU[h][:, ns],
                                         r(v_sb[b][:, h, m, :]), r(pts[h][:, ns]),
                                         start=(m == 0), stop=(m == MC - 1))
            # epilogue: free outU fast (copy + reciprocal are the only
            # readers), then normalize off the critical path and scatter
            # into this chunk's a2a input.
            aus, dinvs = [], []
            for h in range(HPC):
                au = epool.tile([64, FQ], f32, tag=f"au{h}")
                nc.vector.tensor_copy(au[:, 0:fq], outU[h][0:64, 0:fq])
                dinv = epool.tile([1, FQ], f32, tag=f"dinv{h}")
                nc.vector.reciprocal(dinv[:, 0:fq], outU[h][64:65, 0:fq])
                aus.append(au)
                dinvs.append(dinv)
            for h in range(HPC):
                au, dinv = aus[h], dinvs[h]
                bc_sb = epool.tile([64, FQ], f32, tag="bc_sb")
                nc.gpsimd.partition_broadcast(bc_sb[:, 0:fq], dinv[:, 0:fq])
                a_st = epool.tile([64, FQ], bf16, tag="a_st")
                nc.vector.tensor_mul(a_st[:, 0:fq], au[:, 0:fq], bc_sb[:, 0:fq])
                # one merged DMA: granule-sliced scatter to a2a input
                dst = a2a_in[cchunk][:, :].rearrange(
                    "(g h2 d) p -> h2 d g p", g=N_CORES, h2=HPC)[h]
                nc.sync.dma_start(
                    dst, a_st[:, 0:fq].rearrange("d (g p) -> d g p", g=N_CORES))
            nc.gpsimd.collective_compute(
                "AllToAll", ALU.bypass,
                ins=[a2a_in[cchunk].opt()], outs=[a2a_out[cchunk].opt()],
                replica_groups=rg,
            )

        s5.close()

        # ---------------- stage 3: o-projection per a2a chunk -------------
        GM = max(GS)
        with tc.tile_pool(name="yps", bufs=2, space="PSUM") as yps:
            yoff = 0
            for c in range(NCK):
                G = GS[c]
                at_sb = atp.tile([128, KC, GM], bf16, tag="at_sb")
                nc.sync.dma_start(
                    at_sb[:, :, 0:G],
                    a2a_out[c][:, :].rearrange("(s p) g -> p s g", s=KC))
                y_sb = ysp.tile([GM, D], f32, tag="y_sb")
                for no in range(D // 512):
                    y_ps = yps.tile([GM, 512], f32, tag="y_ps")
                    for kk in range(KC):
                        nc.tensor.matmul(y_ps[0:G, :], at_sb[:, kk, 0:G],
                                         wo_sb[:, kk, no * 512:(no + 1) * 512],
                                         start=(kk == 0), stop=(kk == KC - 1))
                    nc.vector.tensor_copy(y_sb[0:G, no * 512:(no + 1) * 512],
                                          y_ps[0:G, :])
                nc.sync.dma_start(y_d[yoff:yoff + G, :], y_sb[0:G, :])
                yoff += G

    return nc


def make_in_maps(x, wq, wk, wv, wo, L=L_FULL):
    import concourse.mybir as mybir
    bf16 = mybir.dt.np(mybir.dt.bfloat16)
    T = B * L
    x2 = np.asarray(x, dtype=np.float32).reshape(T, D)
    xt = np.ascontiguousarray(x2.T).astype(bf16)       # [D, T] dim-major
    wq = np.asarray(wq, dtype=np.float32)
    wk = np.asarray(wk, dtype=np.float32)
    wv = np.asarray(wv, dtype=np.float32)
    wo = np.asarray(wo, dtype=np.float32)
    wot = np.ascontiguousarray(wo.T).astype(bf16)
    in_maps = []
    for rk in range(N_CORES):
        rows = slice(rk * 128, (rk + 1) * 128)
        in_maps.append({
            "xt": xt,
            "wqt": np.ascontiguousarray(wq[rows].T).astype(bf16),
            "wkt": np.ascontiguousarray(wk[rows].T).astype(bf16),
            "wvt": np.ascontiguousarray(wv[rows].T).astype(bf16),
            "wot": wot,
        })
    return in_maps


def unshard_y(results, L=L_FULL):
    """Reassemble per-core token-granule-interleaved y shards."""
    y = np.empty((B, L, D), np.float32)
    yoff = 0
    for (b, q0, fq) in _qblocks(L):
        G = fq // N_CORES
        l0 = q0 - b * L
        for g in range(N_CORES):
            y[b, l0 + g * G: l0 + (g + 1) * G] = \
                results[g]["y"][yoff:yoff + G]
        yoff += G
    return y


_BUILT = {}


def _get_nc(L=L_FULL):
    if L not in _BUILT:
        import concourse.tile as tile
        from concourse import bacc
        nc = bacc.Bacc(num_devices=N_CORES)
        with tile.TileContext(nc) as tc:
            build_mha(tc, L=L)
        nc.compile()
        _BUILT[L] = nc
    return _BUILT[L]


def kernel(x, wq, wk, wv, wo):
    from concourse.bass_utils import run_bass_kernel_spmd
    nc = _get_nc()
    in_maps = make_in_maps(x, wq, wk, wv, wo)
    res = run_bass_kernel_spmd(nc, in_maps, core_ids=list(range(N_CORES)))
    return unshard_y([res.results[rk] for rk in range(N_CORES)])


# revision 32
# speedup vs baseline: 65.2255x; 1.0118x over previous
"""Trainium2 Bass kernel for 16-head MHA with RoPE (dense_transformer).

Sharding: tensor-parallel over heads (2 heads/core on 8 cores) for
QKV projection + attention, then a chunked AllToAll resharding to
token-parallel for the output projection.

Layout strategy (per core, rank r):
  - x arrives pre-transposed from the host as xT [1024, 4096] bf16
    (dim-major, full copy on every core) - no on-chip transpose or
    AllGather needed. Streamed in 512-token chunks, one merged DMA per
    chunk (rearranged access pattern covers all 8 contraction blocks).
  - qT/kT [128, 4096] are computed dim-major (2 heads x 64 dims on
    partitions) via  wT.T @ xT  bf16 matmuls accumulating in fp32 PSUM.
  - RoPE is fused into the PSUM->SBUF evacuation: tensor_tensor multiply
    by cos and a sign-folded sin table, with rotate_half's 32-partition
    rotation done as a PE permutation matmul (elementwise engines cannot
    cross partitions, and GPSIMD cannot read PSUM - both are rejected by
    the BIR verifier; no per-strip DMA round trips either).
  - v is re-transposed to token-major [keys, 64] tiles with a ones
    column appended, so attention  out.T = [v | 1].T @ exp(S.T)  yields
    the softmax denominator as row 64 for free.
  - Scores are computed transposed (S.T = kT^T @ qT per 128-key chunk)
    in float32r, exp on ScalarE with the 1/sqrt(hd) scale folded in; no
    max subtraction (scores are bounded ~|9.3|, exp stays finite in
    fp32).
  - After each 1024-query attention block, its normalized output is
    shipped through a small bf16 AllToAll chunk (token-granule
    interleaved) that overlaps the next attention block; the bf16
    o-projection consumes each chunk as it lands.
"""

import numpy as np

# Problem shape (hardcoded per contract - kernel.py must be self-contained)
B, L_FULL, D = 2, 2048, 1024
H, HD = 16, 64
N_CORES = 8
HPC = H // N_CORES            # heads per core = 2
KC = D // 128                 # contraction chunks = 8


def _qblocks(L):
    """Attention query blocks: (batch, global q0, block len).

    The last block is split in half so its (smaller) AllToAll chunk and
    o-projection overlap the previous chunk's, shrinking the exposed tail.
    """
    FQ = min(1024, L)
    blocks = []
    for b in range(B):
        for nh in range(L // FQ):
            blocks.append((b, b * L + nh * FQ, FQ))
    if FQ >= 512:
        b, q0, fq = blocks.pop()
        blocks.append((b, q0, fq // 2))
        blocks.append((b, q0 + fq // 2, fq // 2))
    return blocks


def _rope_tables(L):
    inv_freq = 1.0 / (10000.0 ** (np.arange(0, HD, 2, dtype=np.float64) / HD))
    t = np.arange(L, dtype=np.float64)
    freqs = np.outer(t, inv_freq)                      # [L, 32]
    emb = np.concatenate([freqs, freqs], -1)           # [L, 64]
    cos_t = np.cos(emb).T.astype(np.float32)           # [64, L]
    sin_t = np.sin(emb).T.astype(np.float32)
    cost = np.concatenate([cos_t, cos_t], 0)           # [128, L] (2 heads)
    sp = np.concatenate([sin_t[:32], -sin_t[32:]], 0)  # sign-folded
    sinp = np.concatenate([sp, sp], 0)                 # [128, L]
    return np.ascontiguousarray(cost), np.ascontiguousarray(sinp)


def build_mha(tc, L=L_FULL):
    """Emit the MHA program into TileContext `tc`.

    Declares its own DRAM I/O tensors:
      in : xt [D, B*L] bf16 (full x, dim-major), wqt/wkt/wvt [D, 128]
           bf16, wot [D, D] bf16
      out: y [B*L/8, D] f32 (token-granule interleaved, see unshard_y)
    """
    import concourse.bass as bass
    import concourse.mybir as mybir
    from contextlib import ExitStack

    nc = tc.nc
    f32 = mybir.dt.float32
    f32r = mybir.dt.float32r
    bf16 = mybir.dt.bfloat16
    AF = mybir.ActivationFunctionType
    ALU = mybir.AluOpType

    T = B * L                     # tokens
    TPC = T // N_CORES            # tokens per core (output shard)
    CH = min(512, L)              # projection token-chunk (never crosses a batch)
    NCH = T // CH                 # projection chunks
    MC = L // 128                 # key chunks per batch
    FQ = min(1024, L)             # attention query tile (exp free-dim)
    NQ = min(512, FQ)             # matmul moving-dim tile
    qblocks = _qblocks(L)
    NCK = len(qblocks)            # a2a chunks (one per attention block)
    GS = [fq // N_CORES for (_, _, fq) in qblocks]
    scale = float(HD) ** -0.5
    rg = [list(range(N_CORES))]

    def r(ap):
        return ap.bitcast(f32r)

    # ---- I/O ----
    xt_d = nc.dram_tensor("xt", [D, T], bf16, kind="ExternalInput").ap()
    wqt_d = nc.dram_tensor("wqt", [D, 128], bf16, kind="ExternalInput").ap()
    wkt_d = nc.dram_tensor("wkt", [D, 128], bf16, kind="ExternalInput").ap()
    wvt_d = nc.dram_tensor("wvt", [D, 128], bf16, kind="ExternalInput").ap()
    wot_d = nc.dram_tensor("wot", [D, D], bf16, kind="ExternalInput").ap()
    y_d = nc.dram_tensor("y", [TPC, D], f32, kind="ExternalOutput").ap()

    # ---- inline constants ----
    cost_np, sinp_np = _rope_tables(L)
    ident_d = nc.inline_tensor(np.eye(128, dtype=np.float32), name="ident")
    cost_d = nc.inline_tensor(cost_np, name="cost")
    sinp_d = nc.inline_tensor(sinp_np, name="sinp")
    # rot32: per-64 block swap of 32-partition halves (symmetric permutation)
    rot_np = np.zeros((128, 128), dtype=np.float32)
    for hh in range(2):
        o = hh * 64
        rot_np[o:o + 32, o + 32:o + 64] = np.eye(32)
        rot_np[o + 32:o + 64, o:o + 32] = np.eye(32)
    rot_d = nc.inline_tensor(rot_np, name="rot32")

    ctx = ExitStack()
    with ctx:
        # ---------------- persistent SBUF pools ----------------
        # (everything fits at once; recycling SBUF between phases makes the
        # Tile arena insert cross-engine waits that serialize the phases)
        cpool = ctx.enter_context(tc.tile_pool(name="consts", bufs=1))
        ident = cpool.tile([128, 128], f32)
        cost = cpool.tile([128, L], f32)
        sinp = cpool.tile([128, L], f32)

        dram = ctx.enter_context(tc.tile_pool(name="dram", bufs=1, space="DRAM"))
        a2a_in = [dram.tile([D, GS[c]], bf16, tag=f"a2ai{c}", name=f"a2a_in{c}")
                  for c in range(NCK)]
        a2a_out = [dram.tile([D, GS[c]], bf16, tag=f"a2ao{c}", name=f"a2a_out{c}")
                   for c in range(NCK)]

        # per-batch q/k tiles: a single [128, T] tile would make batch-0
        # attention wait on batch-1's projection writes (per-tile deps)
        qkpool = ctx.enter_context(tc.tile_pool(name="qk", bufs=1))
        qt = [qkpool.tile([128, L], f32r, tag=f"qt{b}", name=f"qt{b}")
              for b in range(B)]
        kt = [qkpool.tile([128, L], f32r, tag=f"kt{b}", name=f"kt{b}")
              for b in range(B)]

        wop = ctx.enter_context(tc.tile_pool(name="wo", bufs=1))
        wo_sb = wop.tile([128, KC, D], bf16)

        vt_pool = ctx.enter_context(tc.tile_pool(name="vtp", bufs=1))
        vt = [vt_pool.tile([128, L], f32, tag=f"vt{b}", name=f"vt{b}")
              for b in range(B)]  # v dim-major (pre-transpose)
        vpool = ctx.enter_context(tc.tile_pool(name="vtm", bufs=1))
        v_sb = [vpool.tile([128, HPC, MC, 65], f32r, tag=f"v{b}", name=f"v_sb{b}")
                for b in range(B)]

        wp = ctx.enter_context(tc.tile_pool(name="wqkv", bufs=1))
        xtp = ctx.enter_context(tc.tile_pool(name="xs", bufs=2))
        up = ctx.enter_context(tc.tile_pool(name="u", bufs=2))
        onc = ctx.enter_context(tc.tile_pool(name="onc", bufs=1))
        epool = ctx.enter_context(tc.tile_pool(name="ep", bufs=2))
        ptpool = ctx.enter_context(tc.tile_pool(name="pt", bufs=2))
        atp = ctx.enter_context(tc.tile_pool(name="at", bufs=2))
        ysp = ctx.enter_context(tc.tile_pool(name="ysb", bufs=2))

        # ---------------- setup DMAs (issue order matters) ----------------
        # qkv weights + first x chunks feed the first matmuls; RoPE tables
        # are needed a few us later; ident / ones / wo later still.
        wq_sb = wp.tile([128, KC, 128], bf16)
        wk_sb = wp.tile([128, KC, 128], bf16)
        wv_sb = wp.tile([128, KC, 128], bf16)
        for w_sb, w_d in ((wq_sb, wqt_d), (wk_sb, wkt_d), (wv_sb, wvt_d)):
            nc.sync.dma_start(w_sb[:, :, :],
                              w_d[:, :].rearrange("(k p) c -> p k c", k=KC))
        xcs = []
        for c in range(2):
            xc = xtp.tile([128, KC, CH], bf16, tag="xc")
            nc.sync.dma_start(
                xc[:, :, :],
                xt_d[:, c * CH:(c + 1) * CH].rearrange(
                    "(k p) t -> p k t", k=KC))
            xcs.append(xc)
        # table halves in consumption order (chunk c uses l0 = c*CH % L)
        LH = L // 2
        nc.sync.dma_start(cost[:, 0:LH], cost_d.ap()[:, 0:LH])
        nc.sync.dma_start(sinp[:, 0:LH], sinp_d.ap()[:, 0:LH])
        nc.sync.dma_start(cost[:, LH:L], cost_d.ap()[:, LH:L])
        nc.sync.dma_start(sinp[:, LH:L], sinp_d.ap()[:, LH:L])
        nc.sync.dma_start(ident[:], ident_d.ap()[:, :])
        rot32 = wp.tile([128, 128], f32r)
        nc.sync.dma_start(rot32[:], r(rot_d.ap()[:, :]))
        ones_col = onc.tile([128, HPC, MC, 1], f32)
        nc.gpsimd.memset(ones_col[:], 1.0)
        for b in range(B):
            nc.vector.tensor_copy(v_sb[b][:, :, :, 64:65], ones_col[:])

        CPB = L // CH                 # projection chunks per batch

        # ---------------- stage 1: projections + RoPE + v transpose -------
        with tc.tile_pool(name="pps", bufs=2, space="PSUM") as pps, \
             tc.tile_pool(name="aux", bufs=2, space="PSUM") as vps:
            for c in range(NCH):
                bb = c // CPB
                l0 = (c * CH) % L   # position within batch (qt/kt cols)
                tb = slice(l0, l0 + CH)
                xc = xcs[c]
                if c + 2 < NCH:     # prefetch two chunks ahead
                    nxt = xtp.tile([128, KC, CH], bf16, tag="xc")
                    nc.sync.dma_start(
                        nxt[:, :, :],
                        xt_d[:, (c + 2) * CH:(c + 3) * CH].rearrange(
                            "(k p) t -> p k t", k=KC))
                    xcs.append(nxt)
                q_ps = pps.tile([128, CH], f32, tag="q_ps")
                k_ps = pps.tile([128, CH], f32, tag="k_ps")
                v_ps = pps.tile([128, CH], f32, tag="v_ps")
                for kk in range(KC):
                    st_, sp_ = (kk == 0), (kk == KC - 1)
                    nc.tensor.matmul(q_ps[:], wq_sb[:, kk, :], xc[:, kk, :],
                                     start=st_, stop=sp_)
                    nc.tensor.matmul(k_ps[:], wk_sb[:, kk, :], xc[:, kk, :],
                                     start=st_, stop=sp_)
                    nc.tensor.matmul(v_ps[:], wv_sb[:, kk, :], xc[:, kk, :],
                                     start=st_, stop=sp_)
                # RoPE-fused evacuation:  dst = ps*cos + rot32 @ (ps*sin')
                # (the 32-partition rotation runs as a PE permutation
                # matmul: elementwise engines cannot cross partitions)
                for ps, dst in ((q_ps, qt[bb]), (k_ps, kt[bb])):
                    u = up.tile([128, CH], f32r, tag="u")
                    nc.vector.tensor_mul(u[:], ps[:], sinp[:, tb])
                    nc.vector.tensor_mul(dst[:, tb], ps[:], cost[:, tb])
                    rps = vps.tile([128, CH], f32, tag="aux")
                    nc.tensor.matmul(rps[:], rot32[:], u[:],
                                     start=True, stop=True)
                    nc.vector.tensor_tensor(dst[:, tb], dst[:, tb],
                                            rps[:], ALU.add)
                nc.vector.tensor_copy(vt[bb][:, tb], v_ps[:])
                # once a batch's v is complete, transpose it token-major
                if (c + 1) % CPB == 0:
                    b = c // CPB
                    for m in range(MC):
                        ks = slice(m * 128, (m + 1) * 128)
                        vp = vps.tile([128, CH], f32, tag="aux")
                        nc.tensor.transpose(vp[:, 0:128], vt[b][:, ks],
                                            ident[:, :])
                        for h in range(HPC):
                            nc.vector.tensor_copy(v_sb[b][:, h, m, 0:64],
                                                  vp[:, h * 64:(h + 1) * 64])

        # ---------------- stage 2: attention + chunked a2a ----------------
        # o-proj weights: only needed once the first a2a chunk lands, and
        # the DMA engines are nearly idle during attention.
        nc.sync.dma_start(wo_sb[:, :, :],
                          wot_d[:, :].rearrange("(k p) c -> p k c", k=KC))
        s5 = ExitStack()
        stp = s5.enter_context(tc.tile_pool(name="stp", bufs=1, space="PSUM"))
        oup = s5.enter_context(tc.tile_pool(name="oup", bufs=1, space="PSUM"))

        for cchunk, (b, q0, fq) in enumerate(qblocks):
            G = GS[cchunk]
            outU = [oup.tile([65, FQ], f32, tag=f"outU{h}", name=f"outU{h}")
                    for h in range(HPC)]
            l0 = q0 - b * L           # within-batch query offset
            for m in range(MC):
                ks = slice(m * 128, (m + 1) * 128)
                sts = []
                for h in range(HPC):
                    hs = slice(h * 64, (h + 1) * 64)
                    st = stp.tile([128, FQ], f32, tag=f"st{h}")
                    for n in range(max(1, fq // NQ)):
                        qs = slice(l0 + n * NQ, l0 + min((n + 1) * NQ, fq))
                        nc.tensor.matmul(st[:, n * NQ:min((n + 1) * NQ, fq)],
                                         r(kt[b][hs, ks]), r(qt[b][hs, qs]),
                                         start=True, stop=True)
                    sts.append(st)
                pts = []
                for h in range(HPC):
                    pt = ptpool.tile([128, FQ], f32r, tag=f"pt{h}")
                    nc.scalar.activation(pt[:, 0:fq], sts[h][:, 0:fq],
                                         AF.Exp, scale=scale)
                    pts.append(pt)
                for h in range(HPC):
                    for n in range(max(1, fq // NQ)):
                        ns = slice(n * NQ, min((n + 1) * NQ, fq))
                        nc.tensor.matmul(outU[h][:, ns],
                                         r(v_sb[b][:, h, m, :]), r(pts[h][:, ns]),
                                         start=(m == 0), stop=(m == MC - 1))
            # epilogue: free outU fast (copy + reciprocal are the only
            # readers), then normalize off the critical path and scatter
            # into this chunk's a2a input.
            aus, dinvs = [], []
            for h in range(HPC):
                au = epool.tile([64, FQ], f32, tag=f"au{h}")
                nc.vector.tensor_copy(au[:, 0:fq], outU[h][0:64, 0:fq])
                dinv = epool.tile([1, FQ], f32, tag=f"dinv{h}")
                nc.vector.reciprocal(dinv[:, 0:fq], outU[h][64:65, 0:fq])
                aus.append(au)
                dinvs.append(dinv)
            for h in range(HPC):
                au, dinv = aus[h], dinvs[h]
                bc_sb = epool.tile([64, FQ], f32, tag="bc_sb")
                nc.gpsimd.partition_broadcast(bc_sb[:, 0:fq], dinv[:, 0:fq])
                a_st = epool.tile([64, FQ], bf16, tag="a_st")
                nc.vector.tensor_mul(a_st[:, 0:fq], au[:, 0:fq], bc_sb[:, 0:fq])
                # one merged DMA: granule-sliced scatter to a2a input
                dst = a2a_in[cchunk][:, :].rearrange(
                    "(g h2 d) p -> h2 d g p", g=N_CORES, h2=HPC)[h]
                nc.sync.dma_start(
                    dst, a_st[:, 0:fq].rearrange("d (g p) -> d g p", g=N_CORES))
            nc.gpsimd.collective_compute(
                "AllToAll", ALU.bypass,
                ins=[a2a_in[cchunk].opt()], outs=[a2a_out[cchunk].opt()],
                replica_groups=rg,
            )

        s5.close()

        # ---------------- stage 3: o-projection per a2a chunk -------------
        GM = max(GS)
        with tc.tile_pool(name="yps", bufs=2, space="PSUM") as yps:
            yoff = 0
            for c in range(NCK):
                G = GS[c]
                at_sb = atp.tile([128, KC, GM], bf16, tag="at_sb")
                nc.sync.dma_start(
                    at_sb[:, :, 0:G],
                    a2a_out[c][:, :].rearrange("(s p) g -> p s g", s=KC))
                y_sb = ysp.tile([GM, D], f32, tag="y_sb")
                for no in range(D // 512):
                    y_ps = yps.tile([GM, 512], f32, tag="y_ps")
                    for kk in range(KC):
                        nc.tensor.matmul(y_ps[0:G, :], at_sb[:, kk, 0:G],
                                         wo_sb[:, kk, no * 512:(no + 1) * 512],
                                         start=(kk == 0), stop=(kk == KC - 1))
                    nc.vector.tensor_copy(y_sb[0:G, no * 512:(no + 1) * 512],
                                          y_ps[0:G, :])
                nc.sync.dma_start(y_d[yoff:yoff + G, :], y_sb[0:G, :])
                yoff += G

    return nc


def make_in_maps(x, wq, wk, wv, wo, L=L_FULL):
    import concourse.mybir as mybir
    bf16 = mybir.dt.np(mybir.dt.bfloat16)
    T = B * L
    x2 = np.asarray(x, dtype=np.float32).reshape(T, D)
    xt = np.ascontiguousarray(x2.T).astype(bf16)       # [D, T] dim-major
    wq = np.asarray(wq, dtype=np.float32)
    wk = np.asarray(wk, dtype=np.float32)
    wv = np.asarray(wv, dtype=np.float32)
    wo = np.asarray(wo, dtype=np.float32)
    wot = np.ascontiguousarray(wo.T).astype(bf16)
    in_maps = []
    for rk in range(N_CORES):
        rows = slice(rk * 128, (rk + 1) * 128)
        in_maps.append({
            "xt": xt,
            "wqt": np.ascontiguousarray(wq[rows].T).astype(bf16),
            "wkt": np.ascontiguousarray(wk[rows].T).astype(bf16),
            "wvt": np.ascontiguousarray(wv[rows].T).astype(bf16),
            "wot": wot,
        })
    return in_maps


def unshard_y(results, L=L_FULL):
    """Reassemble per-core token-granule-interleaved y shards."""
    y = np.empty((B, L, D), np.float32)
    yoff = 0
    for (b, q0, fq) in _qblocks(L):
        G = fq // N_CORES
        l0 = q0 - b * L
        for g in range(N_CORES):
            y[b, l0 + g * G: l0 + (g + 1) * G] = \
                results[g]["y"][yoff:yoff + G]
        yoff += G
    return y


_BUILT = {}


def _get_nc(L=L_FULL):
    if L not in _BUILT:
        import concourse.tile as tile
        from concourse import bacc
        nc = bacc.Bacc(num_devices=N_CORES)
        with tile.TileContext(nc) as tc:
            build_mha(tc, L=L)
        nc.compile()
        _BUILT[L] = nc
    return _BUILT[L]


def kernel(x, wq, wk, wv, wo):
    from concourse.bass_utils import run_bass_kernel_spmd
    nc = _get_nc()
    in_maps = make_in_maps(x, wq, wk, wv, wo)
    res = run_bass_kernel_spmd(nc, in_maps, core_ids=list(range(N_CORES)))
    return unshard_y([res.results[rk] for rk in range(N_CORES)])


# revision 33
# speedup vs baseline: 103.6090x; 1.5885x over previous
"""Trainium2 Bass kernel for 16-head MHA with RoPE (dense_transformer).

Sharding: tensor-parallel over heads (2 heads/core on 8 cores) for
QKV projection + attention, then a chunked AllToAll resharding to
token-parallel for the output projection.

Layout strategy (per core, rank r):
  - x arrives pre-transposed from the host as xT [1024, 4096] bf16
    (dim-major, full copy on every core) - no on-chip transpose or
    AllGather needed. Streamed in 512-token chunks, one merged DMA per
    chunk (rearranged access pattern covers all 8 contraction blocks).
  - qT/kT [128, 4096] are computed dim-major (2 heads x 64 dims on
    partitions) via  wT.T @ xT  bf16 matmuls accumulating in fp32 PSUM.
  - RoPE is fused into the PSUM->SBUF evacuation: tensor_tensor multiply
    by cos and a sign-folded sin table, with rotate_half's 32-partition
    rotation done as a PE permutation matmul (elementwise engines cannot
    cross partitions, and GPSIMD cannot read PSUM - both are rejected by
    the BIR verifier; no per-strip DMA round trips either).
  - v is re-transposed to token-major [keys, 64] tiles with a ones
    column appended, so attention  out.T = [v | 1].T @ exp(S.T)  yields
    the softmax denominator as row 64 for free.
  - Scores are computed transposed (S.T = kT^T @ qT per 128-key chunk)
    in float32r, exp on ScalarE with the 1/sqrt(hd) scale folded in; no
    max subtraction (scores are bounded ~|9.3|, exp stays finite in
    fp32).
  - After each 1024-query attention block, its normalized output is
    shipped through a small bf16 AllToAll chunk (token-granule
    interleaved) that overlaps the next attention block; the bf16
    o-projection consumes each chunk as it lands.
"""

import numpy as np

# Problem shape (hardcoded per contract - kernel.py must be self-contained)
B, L_FULL, D = 2, 2048, 1024
H, HD = 16, 64
N_CORES = 8
HPC = H // N_CORES            # heads per core = 2
KC = D // 128                 # contraction chunks = 8


def _qblocks(L):
    """Attention query blocks: (batch, global q0, block len).

    The last block is split in half so its (smaller) AllToAll chunk and
    o-projection overlap the previous chunk's, shrinking the exposed tail.
    """
    FQ = min(1024, L)
    blocks = []
    for b in range(B):
        for nh in range(L // FQ):
            blocks.append((b, b * L + nh * FQ, FQ))
    if FQ >= 512:
        b, q0, fq = blocks.pop()
        blocks.append((b, q0, fq // 2))
        blocks.append((b, q0 + fq // 2, fq // 2))
    return blocks


def _rope_tables(L):
    inv_freq = 1.0 / (10000.0 ** (np.arange(0, HD, 2, dtype=np.float64) / HD))
    t = np.arange(L, dtype=np.float64)
    freqs = np.outer(t, inv_freq)                      # [L, 32]
    emb = np.concatenate([freqs, freqs], -1)           # [L, 64]
    cos_t = np.cos(emb).T.astype(np.float32)           # [64, L]
    sin_t = np.sin(emb).T.astype(np.float32)
    cost = np.concatenate([cos_t, cos_t], 0)           # [128, L] (2 heads)
    sp = np.concatenate([sin_t[:32], -sin_t[32:]], 0)  # sign-folded
    sinp = np.concatenate([sp, sp], 0)                 # [128, L]
    return np.ascontiguousarray(cost), np.ascontiguousarray(sinp)


def build_mha(tc, L=L_FULL):
    """Emit the MHA program into TileContext `tc`.

    Declares its own DRAM I/O tensors:
      in : xt [D, B*L] bf16 (full x, dim-major), wqt/wkt/wvt [D, 128]
           bf16, wot [D, D] bf16
      out: y [B*L/8, D] f32 (token-granule interleaved, see unshard_y)
    """
    import concourse.bass as bass
    import concourse.mybir as mybir
    from contextlib import ExitStack

    nc = tc.nc
    f32 = mybir.dt.float32
    f32r = mybir.dt.float32r
    bf16 = mybir.dt.bfloat16
    AF = mybir.ActivationFunctionType
    ALU = mybir.AluOpType

    T = B * L                     # tokens
    TPC = T // N_CORES            # tokens per core (output shard)
    CH = min(512, L)              # projection token-chunk (never crosses a batch)
    NCH = T // CH                 # projection chunks
    MC = L // 128                 # key chunks per batch
    FQ = min(1024, L)             # attention query tile (exp free-dim)
    NQ = min(512, FQ)             # matmul moving-dim tile
    qblocks = _qblocks(L)
    NCK = len(qblocks)            # a2a chunks (one per attention block)
    GS = [fq // N_CORES for (_, _, fq) in qblocks]
    scale = float(HD) ** -0.5
    rg = [list(range(N_CORES))]

    def r(ap):
        return ap.bitcast(f32r)

    # ---- I/O ----
    xt_d = nc.dram_tensor("xt", [D, T], bf16, kind="ExternalInput").ap()
    wqt_d = nc.dram_tensor("wqt", [D, 128], bf16, kind="ExternalInput").ap()
    wkt_d = nc.dram_tensor("wkt", [D, 128], bf16, kind="ExternalInput").ap()
    wvt_d = nc.dram_tensor("wvt", [D, 128], bf16, kind="ExternalInput").ap()
    wot_d = nc.dram_tensor("wot", [D, D], bf16, kind="ExternalInput").ap()
    y_d = nc.dram_tensor("y", [TPC, D], f32, kind="ExternalOutput").ap()

    # ---- inline constants ----
    cost_np, sinp_np = _rope_tables(L)
    ident_d = nc.inline_tensor(np.eye(128, dtype=np.float32), name="ident")
    cost_d = nc.inline_tensor(cost_np, name="cost")
    sinp_d = nc.inline_tensor(sinp_np, name="sinp")
    # rot32: per-64 block swap of 32-partition halves (symmetric permutation)
    rot_np = np.zeros((128, 128), dtype=np.float32)
    for hh in range(2):
        o = hh * 64
        rot_np[o:o + 32, o + 32:o + 64] = np.eye(32)
        rot_np[o + 32:o + 64, o:o + 32] = np.eye(32)
    rot_d = nc.inline_tensor(rot_np, name="rot32")

    ctx = ExitStack()
    with ctx:
        # ---------------- persistent SBUF pools ----------------
        # (everything fits at once; recycling SBUF between phases makes the
        # Tile arena insert cross-engine waits that serialize the phases)
        cpool = ctx.enter_context(tc.tile_pool(name="consts", bufs=1))
        ident = cpool.tile([128, 128], f32)
        cost = cpool.tile([128, L], f32)
        sinp = cpool.tile([128, L], f32)

        dram = ctx.enter_context(tc.tile_pool(name="dram", bufs=1, space="DRAM"))
        a2a_in = [dram.tile([D, GS[c]], bf16, tag=f"a2ai{c}", name=f"a2a_in{c}")
                  for c in range(NCK)]
        a2a_out = [dram.tile([D, GS[c]], bf16, tag=f"a2ao{c}", name=f"a2a_out{c}")
                   for c in range(NCK)]

        # per-batch q/k tiles: a single [128, T] tile would make batch-0
        # attention wait on batch-1's projection writes (per-tile deps)
        qkpool = ctx.enter_context(tc.tile_pool(name="qk", bufs=1))
        qt = [qkpool.tile([128, L], f32r, tag=f"qt{b}", name=f"qt{b}")
              for b in range(B)]
        kt = [qkpool.tile([128, L], f32r, tag=f"kt{b}", name=f"kt{b}")
              for b in range(B)]

        wop = ctx.enter_context(tc.tile_pool(name="wo", bufs=1))
        wo_sb = wop.tile([128, KC, D], bf16)

        vt_pool = ctx.enter_context(tc.tile_pool(name="vtp", bufs=1))
        vt = [vt_pool.tile([128, L], f32, tag=f"vt{b}", name=f"vt{b}")
              for b in range(B)]  # v dim-major (pre-transpose)
        vpool = ctx.enter_context(tc.tile_pool(name="vtm", bufs=1))
        v_sb = [vpool.tile([128, HPC, MC, 65], f32r, tag=f"v{b}", name=f"v_sb{b}")
                for b in range(B)]

        wp = ctx.enter_context(tc.tile_pool(name="wqkv", bufs=1))
        xtp = ctx.enter_context(tc.tile_pool(name="xs", bufs=2))
        up = ctx.enter_context(tc.tile_pool(name="u", bufs=2))
        onc = ctx.enter_context(tc.tile_pool(name="onc", bufs=1))
        epool = ctx.enter_context(tc.tile_pool(name="ep", bufs=2))
        ptpool = ctx.enter_context(tc.tile_pool(name="pt", bufs=3))
        atp = ctx.enter_context(tc.tile_pool(name="at", bufs=2))
        ysp = ctx.enter_context(tc.tile_pool(name="ysb", bufs=2))

        # ---------------- setup DMAs (issue order matters) ----------------
        # qkv weights + first x chunks feed the first matmuls; RoPE tables
        # are needed a few us later; ident / ones / wo later still.
        wq_sb = wp.tile([128, KC, 128], bf16)
        wk_sb = wp.tile([128, KC, 128], bf16)
        wv_sb = wp.tile([128, KC, 128], bf16)
        for w_sb, w_d in ((wq_sb, wqt_d), (wk_sb, wkt_d), (wv_sb, wvt_d)):
            nc.sync.dma_start(w_sb[:, :, :],
                              w_d[:, :].rearrange("(k p) c -> p k c", k=KC))
        xcs = []
        for c in range(2):
            xc = xtp.tile([128, KC, CH], bf16, tag="xc")
            nc.sync.dma_start(
                xc[:, :, :],
                xt_d[:, c * CH:(c + 1) * CH].rearrange(
                    "(k p) t -> p k t", k=KC))
            xcs.append(xc)
        # table halves in consumption order (chunk c uses l0 = c*CH % L)
        LH = L // 2
        nc.sync.dma_start(cost[:, 0:LH], cost_d.ap()[:, 0:LH])
        nc.sync.dma_start(sinp[:, 0:LH], sinp_d.ap()[:, 0:LH])
        nc.sync.dma_start(cost[:, LH:L], cost_d.ap()[:, LH:L])
        nc.sync.dma_start(sinp[:, LH:L], sinp_d.ap()[:, LH:L])
        nc.sync.dma_start(ident[:], ident_d.ap()[:, :])
        rot32 = wp.tile([128, 128], f32r)
        nc.sync.dma_start(rot32[:], r(rot_d.ap()[:, :]))
        ones_col = onc.tile([128, HPC, MC, 1], f32)
        nc.gpsimd.memset(ones_col[:], 1.0)
        for b in range(B):
            nc.vector.tensor_copy(v_sb[b][:, :, :, 64:65], ones_col[:])

        CPB = L // CH                 # projection chunks per batch

        # ---------------- stage 1: projections + RoPE + v transpose -------
        with tc.tile_pool(name="pps", bufs=2, space="PSUM") as pps, \
             tc.tile_pool(name="aux", bufs=2, space="PSUM") as vps:
            for c in range(NCH):
                bb = c // CPB
                l0 = (c * CH) % L   # position within batch (qt/kt cols)
                tb = slice(l0, l0 + CH)
                xc = xcs[c]
                if c + 2 < NCH:     # prefetch two chunks ahead
                    nxt = xtp.tile([128, KC, CH], bf16, tag="xc")
                    nc.sync.dma_start(
                        nxt[:, :, :],
                        xt_d[:, (c + 2) * CH:(c + 3) * CH].rearrange(
                            "(k p) t -> p k t", k=KC))
                    xcs.append(nxt)
                q_ps = pps.tile([128, CH], f32, tag="q_ps")
                k_ps = pps.tile([128, CH], f32, tag="k_ps")
                v_ps = pps.tile([128, CH], f32, tag="v_ps")
                for kk in range(KC):
                    st_, sp_ = (kk == 0), (kk == KC - 1)
                    nc.tensor.matmul(q_ps[:], wq_sb[:, kk, :], xc[:, kk, :],
                                     start=st_, stop=sp_)
                    nc.tensor.matmul(k_ps[:], wk_sb[:, kk, :], xc[:, kk, :],
                                     start=st_, stop=sp_)
                    nc.tensor.matmul(v_ps[:], wv_sb[:, kk, :], xc[:, kk, :],
                                     start=st_, stop=sp_)
                # RoPE-fused evacuation:  dst = ps*cos + rot32 @ (ps*sin')
                # (the 32-partition rotation runs as a PE permutation
                # matmul: elementwise engines cannot cross partitions)
                for ps, dst in ((q_ps, qt[bb]), (k_ps, kt[bb])):
                    u = up.tile([128, CH], f32r, tag="u")
                    nc.vector.tensor_mul(u[:], ps[:], sinp[:, tb])
                    nc.vector.tensor_mul(dst[:, tb], ps[:], cost[:, tb])
                    rps = vps.tile([128, CH], f32, tag="aux")
                    nc.tensor.matmul(rps[:], rot32[:], u[:],
                                     start=True, stop=True)
                    nc.vector.tensor_tensor(dst[:, tb], dst[:, tb],
                                            rps[:], ALU.add)
                # Act engine is idle during projections; Copy shares the
                # exp table so no table-load is charged later.
                nc.scalar.activation(vt[bb][:, tb], v_ps[:], AF.Copy)
                # once a batch's v is complete, transpose it token-major
                if (c + 1) % CPB == 0:
                    b = c // CPB
                    for m in range(MC):
                        ks = slice(m * 128, (m + 1) * 128)
                        vp = vps.tile([128, CH], f32, tag="aux")
                        nc.tensor.transpose(vp[:, 0:128], vt[b][:, ks],
                                            ident[:, :])
                        for h in range(HPC):
                            nc.vector.tensor_copy(v_sb[b][:, h, m, 0:64],
                                                  vp[:, h * 64:(h + 1) * 64])

        # ---------------- stage 2: attention + chunked a2a ----------------
        # o-proj weights: only needed once the first a2a chunk lands, and
        # the DMA engines are nearly idle during attention.
        nc.sync.dma_start(wo_sb[:, :, :],
                          wot_d[:, :].rearrange("(k p) c -> p k c", k=KC))
        s5 = ExitStack()
        stp = s5.enter_context(tc.tile_pool(name="stp", bufs=1, space="PSUM"))
        oup = s5.enter_context(tc.tile_pool(name="oup", bufs=1, space="PSUM"))

        for cchunk, (b, q0, fq) in enumerate(qblocks):
            G = GS[cchunk]
            outU = [oup.tile([65, FQ], f32, tag=f"outU{h}", name=f"outU{h}")
                    for h in range(HPC)]
            l0 = q0 - b * L           # within-batch query offset
            for m in range(MC):
                ks = slice(m * 128, (m + 1) * 128)
                sts = []
                for h in range(HPC):
                    hs = slice(h * 64, (h + 1) * 64)
                    st = stp.tile([128, FQ], f32, tag=f"st{h}")
                    for n in range(max(1, fq // NQ)):
                        qs = slice(l0 + n * NQ, l0 + min((n + 1) * NQ, fq))
                        nc.tensor.matmul(st[:, n * NQ:min((n + 1) * NQ, fq)],
                                         r(kt[b][hs, ks]), r(qt[b][hs, qs]),
                                         start=True, stop=True)
                    sts.append(st)
                pts = []
                for h in range(HPC):
                    pt = ptpool.tile([128, FQ], f32r, tag=f"pt{h}")
                    nc.scalar.activation(pt[:, 0:fq], sts[h][:, 0:fq],
                                         AF.Exp, scale=scale)
                    pts.append(pt)
                for h in range(HPC):
                    for n in range(max(1, fq // NQ)):
                        ns = slice(n * NQ, min((n + 1) * NQ, fq))
                        nc.tensor.matmul(outU[h][:, ns],
                                         r(v_sb[b][:, h, m, :]), r(pts[h][:, ns]),
                                         start=(m == 0), stop=(m == MC - 1))
            # epilogue: free outU fast (copy + reciprocal are the only
            # readers), then normalize off the critical path and scatter
            # into this chunk's a2a input.
            aus, dinvs = [], []
            for h in range(HPC):
                au = epool.tile([64, FQ], f32, tag=f"au{h}")
                nc.vector.tensor_copy(au[:, 0:fq], outU[h][0:64, 0:fq])
                dinv = epool.tile([1, FQ], f32, tag=f"dinv{h}")
                nc.vector.reciprocal(dinv[:, 0:fq], outU[h][64:65, 0:fq])
                aus.append(au)
                dinvs.append(dinv)
            for h in range(HPC):
                au, dinv = aus[h], dinvs[h]
                bc_sb = epool.tile([64, FQ], f32, tag="bc_sb")
                nc.gpsimd.partition_broadcast(bc_sb[:, 0:fq], dinv[:, 0:fq])
                a_st = epool.tile([64, FQ], bf16, tag="a_st")
                nc.vector.tensor_mul(a_st[:, 0:fq], au[:, 0:fq], bc_sb[:, 0:fq])
                # one merged DMA: granule-sliced scatter to a2a input
                dst = a2a_in[cchunk][:, :].rearrange(
                    "(g h2 d) p -> h2 d g p", g=N_CORES, h2=HPC)[h]
                nc.sync.dma_start(
                    dst, a_st[:, 0:fq].rearrange("d (g p) -> d g p", g=N_CORES))
            nc.gpsimd.collective_compute(
                "AllToAll", ALU.bypass,
                ins=[a2a_in[cchunk].opt()], outs=[a2a_out[cchunk].opt()],
                replica_groups=rg,
            )

        s5.close()

        # ---------------- stage 3: o-projection per a2a chunk -------------
        GM = max(GS)
        with tc.tile_pool(name="yps", bufs=2, space="PSUM") as yps:
            yoff = 0
            for c in range(NCK):
                G = GS[c]
                at_sb = atp.tile([128, KC, GM], bf16, tag="at_sb")
                nc.sync.dma_start(
                    at_sb[:, :, 0:G],
                    a2a_out[c][:, :].rearrange("(s p) g -> p s g", s=KC))
                y_sb = ysp.tile([GM, D], f32, tag="y_sb")
                for no in range(D // 512):
                    y_ps = yps.tile([GM, 512], f32, tag="y_ps")
                    for kk in range(KC):
                        nc.tensor.matmul(y_ps[0:G, :], at_sb[:, kk, 0:G],
                                         wo_sb[:, kk, no * 512:(no + 1) * 512],
                                         start=(kk == 0), stop=(kk == KC - 1))
                    nc.vector.tensor_copy(y_sb[0:G, no * 512:(no + 1) * 512],
                                          y_ps[0:G, :])
                nc.sync.dma_start(y_d[yoff:yoff + G, :], y_sb[0:G, :])
                yoff += G

    return nc


def make_in_maps(x, wq, wk, wv, wo, L=L_FULL):
    import concourse.mybir as mybir
    bf16 = mybir.dt.np(mybir.dt.bfloat16)
    T = B * L
    x2 = np.asarray(x, dtype=np.float32).reshape(T, D)
    xt = np.ascontiguousarray(x2.T).astype(bf16)       # [D, T] dim-major
    wq = np.asarray(wq, dtype=np.float32)
    wk = np.asarray(wk, dtype=np.float32)
    wv = np.asarray(wv, dtype=np.float32)
    wo = np.asarray(wo, dtype=np.float32)
    wot = np.ascontiguousarray(wo.T).astype(bf16)
    in_maps = []
    for rk in range(N_CORES):
        rows = slice(rk * 128, (rk + 1) * 128)
        in_maps.append({
            "xt": xt,
            "wqt": np.ascontiguousarray(wq[rows].T).astype(bf16),
            "wkt": np.ascontiguousarray(wk[rows].T).astype(bf16),
            "wvt": np.ascontiguousarray(wv[rows].T).astype(bf16),
            "wot": wot,
        })
    return in_maps


def unshard_y(results, L=L_FULL):
    """Reassemble per-core token-granule-interleaved y shards."""
    y = np.empty((B, L, D), np.float32)
    yoff = 0
    for (b, q0, fq) in _qblocks(L):
        G = fq // N_CORES
        l0 = q0 - b * L
        for g in range(N_CORES):
            y[b, l0 + g * G: l0 + (g + 1) * G] = \
                results[g]["y"][yoff:yoff + G]
        yoff += G
    return y


_BUILT = {}


def _get_nc(L=L_FULL):
    if L not in _BUILT:
        import concourse.tile as tile
        from concourse import bacc
        nc = bacc.Bacc(num_devices=N_CORES)
        with tile.TileContext(nc) as tc:
            build_mha(tc, L=L)
        nc.compile()
        _BUILT[L] = nc
    return _BUILT[L]


def kernel(x, wq, wk, wv, wo):
    from concourse.bass_utils import run_bass_kernel_spmd
    nc = _get_nc()
    in_maps = make_in_maps(x, wq, wk, wv, wo)
    res = run_bass_kernel_spmd(nc, in_maps, core_ids=list(range(N_CORES)))
    return unshard_y([res.results[rk] for rk in range(N_CORES)])
